# revision 1
# baseline (speedup 1.0000x reference)
"""Compact Bilinear Pooling (count-sketch + circular conv + spatial sum-pool)
as a Trainium2 Bass/Tile kernel, SPMD over 8 NeuronCores.

Math: with sk_i = flat @ S_i (flat: [B*P, C]), the reference computes
    out[b] = sum_{p in sample b} ifft( fft(sk1_p) * fft(sk2_p) ).real
Fold the (constant) sketch matrices into the DFT:  Phi_i = fft(S_i, axis=1),
so fft(sk_i,p) = x_p^T Phi_i.  Because sk are real, only the half spectrum
k = 0..D/2 is needed:
    Shat[b,k]  = sum_p (x_p^T Phi1[:,k]) * (x_p^T Phi2[:,k])
    out[b,d]   = sum_k wk*( Re Shat * cos(2 pi k d/D) - Im Shat * sin(...) )/D
Device pipeline per core (sharded by frequency: 512 of 4096 padded freqs),
all matmuls bf16 (PSUM accumulation in f32):
  1. Ghat tiles [128 pos, 512 freq] via bf16 matmuls.
  2. complex products on DVE -> bf16; per-sample position-reduction via
     transposed +-1 indicator matmuls (out [128 freq, 16 samp], free dim 16)
     accumulating Shat^T directly -- no DMA transpose needed.
  3. inverse DFT transposed: out [128 d, 16 samp] per d-tile (free dim 16)
     against cos/sin slabs already laid out [freq, d]; partial [8064, 16]
     per core; host sums the 8 partials and transposes.
"""

import numpy as np
import ml_dtypes

import concourse.bacc as bacc
import concourse.mybir as mybir
import concourse.tile as tile
from concourse.bass_utils import run_bass_kernel_spmd

# problem dims (hardcoded per spec)
B, C, H, W, D = 16, 512, 14, 14, 8000
P = H * W            # 196 positions per sample
BP = B * P           # 3136
KH = D // 2 + 1      # 4001 half-spectrum frequencies
KPAD = 4096          # padded to 8*512
NCORES = 8
KSL = KPAD // NCORES  # 512 freqs per core
NCC = C // 128        # 4 contraction chunks
NPT = (BP + 127) // 128  # 25 position tiles (24x128 + 64)
NKT = KSL // 128      # 4 k-chunks of the core's freq slice
DPAD = 8064           # 63 * 128 output-dim tiles (8000 padded)
NDT = DPAD // 128     # 63 d tiles
XSPLIT = 1024         # first x piece covers pts 0..7

F32 = mybir.dt.float32
BF16 = mybir.dt.bfloat16


def build_nc():
    nc = bacc.Bacc("TRN2", target_bir_lowering=False, debug=False)
    x_d = nc.dram_tensor("x", [C, BP], BF16, kind="ExternalInput")
    phi_d = nc.dram_tensor("phi", [128, NCC * 4 * KSL], BF16, kind="ExternalInput")
    ind_d = nc.dram_tensor("ind", [128, NPT * 2 * B], BF16, kind="ExternalInput")
    cc_d = nc.dram_tensor("cc", [2, NKT, 128, DPAD], BF16, kind="ExternalInput")
    out_d = nc.dram_tensor("out", [DPAD, B], F32, kind="ExternalOutput")

    with tile.TileContext(nc) as tc:
        with (
            tc.tile_pool(name="phi", bufs=1) as phi_pool,
            tc.tile_pool(name="xin", bufs=1) as x_pool,
            tc.tile_pool(name="ccs", bufs=1) as cc_pool,
            tc.tile_pool(name="bcp", bufs=5) as b_pool,
            tc.tile_pool(name="prd", bufs=6) as prod_pool,
            tc.tile_pool(name="sbf", bufs=1) as s_pool,
            tc.tile_pool(name="stage", bufs=4) as st_pool,
            tc.tile_pool(name="mm", bufs=7, space="PSUM") as mm_psum,
            tc.tile_pool(name="sac", bufs=1, space="PSUM") as s_psum,
        ):
            # ---- PE warmup: ramp the clock through its p-states on dummy
            # matmuls while the first input DMAs are in flight
            warm = phi_pool.tile([128, KSL], BF16, tag="warm")
            nc.vector.memset(warm[:], 0.0)
            wps = mm_psum.tile([128, KSL], F32, tag="mm", name="warmps")
            for w in range(8):
                nc.tensor.matmul(wps[:], lhsT=warm[:, 0:128], rhs=warm[:],
                                 start=True, stop=True, skip_group_check=True)

            # ---- inputs, ordered so pt0 can start ASAP: cc0's first phi
            # m-slice and pt0's x columns land first, then the rest in
            # cc-major order, then x tails / ind / inverse slabs
            phi0 = []
            for m in range(4):
                pm = phi_pool.tile([128, KSL], BF16, tag=f"phi0m{m}", name=f"phi0m{m}")
                nc.sync.dma_start(pm[:], phi_d.ap()[:, m * KSL:(m + 1) * KSL])
                phi0.append(pm)
                if m == 0:
                    x0a = x_pool.tile([128, 128], BF16, tag="x0a")
                    nc.sync.dma_start(x0a[:], x_d.ap()[0:128, 0:128])
            x0b = x_pool.tile([128, XSPLIT - 128], BF16, tag="x0b")
            nc.sync.dma_start(x0b[:], x_d.ap()[0:128, 128:XSPLIT])
            phit, xta, xtb = [None], [None], []
            for cc in range(1, NCC):
                pt_ = phi_pool.tile([128, 4 * KSL], BF16, tag=f"phi{cc}")
                nc.sync.dma_start(pt_[:], phi_d.ap()[:, cc * 4 * KSL:(cc + 1) * 4 * KSL])
                phit.append(pt_)
                ta = x_pool.tile([128, XSPLIT], BF16, tag=f"xa{cc}")
                nc.sync.dma_start(ta[:], x_d.ap()[cc * 128:(cc + 1) * 128, 0:XSPLIT])
                xta.append(ta)
            for cc in range(NCC):
                tb = x_pool.tile([128, BP - XSPLIT], BF16, tag=f"xb{cc}")
                nc.sync.dma_start(tb[:], x_d.ap()[cc * 128:(cc + 1) * 128, XSPLIT:])
                xtb.append(tb)
            indt = phi_pool.tile([128, NPT * 2 * B], BF16, tag="ind")
            nc.sync.dma_start(indt[:], ind_d.ap())

            def phi_slice(cc, m):
                if cc == 0:
                    return phi0[m][:]
                return phit[cc][:, m * KSL:(m + 1) * KSL]

            def x_slice(cc, pt, M):
                if pt * 128 + M <= XSPLIT:
                    if cc == 0:
                        if pt == 0:
                            return x0a[:]
                        return x0b[:, (pt - 1) * 128:(pt - 1) * 128 + M]
                    return xta[cc][:, pt * 128:pt * 128 + M]
                o = pt * 128 - XSPLIT
                return xtb[cc][:, o:o + M]
            cct = {}
            for t in range(2):
                for kt in range(NKT):
                    ct = cc_pool.tile([128, DPAD], BF16, tag=f"cc{t}{kt}")
                    nc.sync.dma_start(ct[:], cc_d.ap()[t, kt])
                    cct[(t, kt)] = ct

            # ---- Shat^T accumulator: cols (kt, half, b); half 0=Re, 1=Im
            s_acc = s_psum.tile([128, NKT * 2 * B], F32, tag="sacc")

            # ---- main stage
            for pt in range(NPT):
                M = min(128, BP - pt * 128)
                g = [mm_psum.tile([128, KSL], F32, tag="mm", name=f"g{pt}_{m}")
                     for m in range(4)]
                for cc in range(NCC):
                    xs = x_slice(cc, pt, M)
                    for m in range(4):  # 0:g1re 1:g1im 2:g2re 3:g2im
                        nc.tensor.matmul(
                            g[m][0:M, :],
                            lhsT=xs,
                            rhs=phi_slice(cc, m),
                            start=(cc == 0),
                            stop=(cc == NCC - 1),
                        )
                # copy all four g tiles PSUM->SBUF bf16 on ACT; the DVE
                # products then run all-SBUF/bf16 (fast mode) and the tail
                # drain after the last position tile shrinks
                gb = []
                for m in range(4):
                    t_ = b_pool.tile([128, KSL], BF16, tag=f"gb{m}",
                                     name=f"gb{pt}_{m}")
                    nc.scalar.copy(t_[0:M, :], g[m][0:M, :])
                    gb.append(t_)
                prods = []
                for in0, in1 in ((gb[0], gb[2]), (gb[1], gb[3]),
                                 (gb[0], gb[3]), (gb[1], gb[2])):
                    pr = prod_pool.tile([128, KSL], BF16, tag="prod")
                    nc.vector.tensor_mul(pr[0:M, :], in0[0:M, :], in1[0:M, :])
                    prods.append(pr)

                # per-sample reduce over positions, transposed:
                # s_acc[k, (kt,half,b)] += prod[p, k]^T @ ind[p, b]
                # i=0: RR(+)->Re  i=1: II(-)->Re  i=2: RI(+)->Im  i=3: IR(+)->Im
                for i, (pr, pat, half) in enumerate((
                    (prods[0], 0, 0), (prods[1], 1, 0),
                    (prods[2], 0, 1), (prods[3], 0, 1),
                )):
                    ic = (pt * 2 + pat) * B
                    for kt in range(NKT):
                        sc = kt * 2 * B + half * B
                        # one start/stop per PSUM bank: start=True zeroes the
                        # whole 2KB region, so only the first matmul into the
                        # bank may carry it
                        nc.tensor.matmul(
                            s_acc[:, sc:sc + B],
                            lhsT=pr[0:M, kt * 128:(kt + 1) * 128],
                            rhs=indt[0:M, ic:ic + B],
                            start=(pt == 0 and i == 0 and kt == 0),
                            stop=(pt == NPT - 1 and i == 3 and kt == NKT - 1),
                            skip_group_check=True,
                        )

            # ---- Shat^T -> bf16 SBUF (no transpose needed); split ACT/DVE
            sT = []
            for kt in range(NKT):
                t_ = s_pool.tile([128, 2 * B], BF16, tag=f"sT{kt}")
                nc.scalar.copy(t_[:], s_acc[:, kt * 2 * B:(kt + 1) * 2 * B])
                sT.append(t_)

            # ---- inverse DFT transposed: out[d, b] = sum_k C[k,d]*S[k,b]
            # groups rotate through the mm pool's banks (stage 1 is done);
            # tiny last group keeps the copy+DMA+sem tail off the critical path
            groups = [(0, 21), (21, 21), (42, 20), (62, 1)]
            for d0, nd in groups:
                pinv = mm_psum.tile([128, nd * B], F32, tag="mm", name=f"pinv{d0}")
                for i in range(nd):
                    dt = d0 + i
                    for idx in range(8):
                        t, kt = idx // 4, idx % 4
                        nc.tensor.matmul(
                            pinv[:, i * B:(i + 1) * B],
                            lhsT=cct[(t, kt)][:, dt * 128:(dt + 1) * 128],
                            rhs=sT[kt][:, t * B:(t + 1) * B],
                            start=(i == 0 and idx == 0),
                            stop=(i == nd - 1 and idx == 7),
                            skip_group_check=True,
                        )
                stage = st_pool.tile([128, nd * B], F32, tag="stage", name=f"st{d0}")
                nc.scalar.copy(stage[:, 0:nd * B], pinv[:, 0:nd * B])
                # spread the out-DMAs over different engine queues so their
                # dispatch does not serialize on the SP sequencer
                eng = {0: nc.sync, 21: nc.scalar, 42: nc.sync, 62: nc.scalar}[d0]
                eng.dma_start(
                    out_d.ap()[d0 * 128:(d0 + nd) * 128, :]
                         .rearrange("(dt p) b -> p dt b", p=128),
                    stage[:, 0:nd * B].rearrange("p (dt b) -> p dt b", b=B),
                )

    nc.compile()
    return nc


def make_constants(S1, S2):
    """Host-side constant prep from the sketch matrices (per-core slices)."""
    Phi = np.zeros((4, C, KPAD), np.float32)
    for i, S in enumerate((S1, S2)):
        F = np.fft.fft(S.astype(np.float64), axis=1)[:, :KH]
        Phi[2 * i, :, :KH] = F.real.astype(np.float32)
        Phi[2 * i + 1, :, :KH] = F.imag.astype(np.float32)

    k = np.arange(KPAD, dtype=np.float64)
    wk = np.where((k == 0) | (k == D // 2), 1.0, 2.0) / D
    wk[KH:] = 0.0
    ang = 2.0 * np.pi * np.outer(k, np.arange(D, dtype=np.float64)) / D
    Cst = np.zeros((2, KPAD, DPAD), np.float32)
    Cst[0, :, :D] = wk[:, None] * np.cos(ang)
    Cst[1, :, :D] = -wk[:, None] * np.sin(ang)
    Cst = Cst.astype(ml_dtypes.bfloat16)  # [2, KPAD, DPAD]

    # phi_packed[j]: [128, (cc, m, kk)] = Phi[m, cc*128+p, 512j+kk]
    arr = Phi.reshape(4, NCC, 128, NCORES, KSL)  # [m, cc, p, j, kk]
    phis, ccs = [], []
    for j in range(NCORES):
        a = arr[:, :, :, j]                      # [m, cc, p, kk]
        a = np.ascontiguousarray(np.transpose(a, (1, 0, 2, 3)))  # [cc, m, p, kk]
        phis.append(np.ascontiguousarray(
            a.transpose(2, 0, 1, 3).reshape(128, NCC * 4 * KSL)
        ).astype(ml_dtypes.bfloat16))
        c = Cst.reshape(2, NCORES, NKT, 128, DPAD)[:, j]  # [2, kt, 128, DPAD]
        ccs.append(np.ascontiguousarray(c))

    # indicators: [128, (pt, pat, b)]; pat 0 = +1, pat 1 = -1
    ind = np.zeros((128, NPT * 2 * B), np.float32)
    for pt in range(NPT):
        for r in range(min(128, BP - pt * 128)):
            b = (pt * 128 + r) // P
            ind[r, (pt * 2 + 0) * B + b] = 1.0
            ind[r, (pt * 2 + 1) * B + b] = -1.0
    return phis, ccs, ind.astype(ml_dtypes.bfloat16)


_CACHE = {}


def kernel(x, S1, S2):
    x = np.asarray(x)
    if "k" not in _CACHE:
        phis, ccs, ind = make_constants(np.asarray(S1), np.asarray(S2))
        _CACHE["k"] = (build_nc(), phis, ccs, ind)
    nc, phis, ccs, ind = _CACHE["k"]

    # [B, C, H, W] -> [C, B*P] bf16, row-contiguous for wide DMA lines
    xr = np.ascontiguousarray(
        x.reshape(B, C, P).transpose(1, 0, 2).reshape(C, BP)
    ).astype(ml_dtypes.bfloat16)
    in_maps = [
        {"x": xr, "phi": phis[j], "ind": ind, "cc": ccs[j]}
        for j in range(NCORES)
    ]
    res = run_bass_kernel_spmd(nc, in_maps, list(range(NCORES)))
    out = np.zeros((DPAD, B), np.float32)
    for r in res.results:
        out += r["out"]
    return np.ascontiguousarray(out[:D].T).astype(x.dtype)



# revision 9
# speedup vs baseline: 1.0441x; 1.0441x over previous
"""Compact Bilinear Pooling (count-sketch + circular conv + spatial sum-pool)
as a Trainium2 Bass/Tile kernel, SPMD over 8 NeuronCores.

Math: with sk_i = flat @ S_i (flat: [B*P, C]), the reference computes
    out[b] = sum_{p in sample b} ifft( fft(sk1_p) * fft(sk2_p) ).real
Fold the sketch matrices into the DFT (Phi_i = fft(S_i, axis=1), half
spectrum k = 0..D/2 suffices since sk are real):
    G_m[p,k]  = x_p^T Phi_m[:,k]          (m: g1re g1im g2re g2im)
    Shat[b,k] = mean_{p in b} (G1 G2)[p,k]    (complex product, /196)
    out[b,d]  = sum_k 196*wk*(Re Shat * cos(2 pi k d/D) - Im Shat * sin)/D

v2 design (freqs on partitions, positions on the free dim):
  - 8-way FREQUENCY sharding: core j owns k in [512j, 512j+512) of the
    4096-padded half spectrum; host sums per-core partials.
  - Stage 1 on PE in fp8e4m3 DoubleRow (2 contraction chunks / instr at
    0.5 cyc/row): 3-term split x*Phi ~= x8@p8 + xlo16@p8s + x8s@plo16
    keeps bf16-level accuracy at 3/4 the bf16 matmul cost.
    Per unit (kt freq-tile, sample): out g[128k, 4m, 196pos] psum.
  - ACT copies g psum->sbuf bf16; DVE forms the 4 cross-products with
    two strided muls; Pool (gpsimd) combines Re=RR-II, Im=RI+IR; DVE
    pool_avg reduces positions per sample -> Shat[128k, 2, 16b] bf16.
  - Stage 3 (inverse half-DFT, d and D-d folded): A = ccRe^T Sre,
    B = ccIm^T Sim accumulate in 2 psum banks across all kt
    (interleaved with stage 1); host: out[d]=A+B, out[D-d]=A-B.
"""

import numpy as np
import ml_dtypes

import concourse.bacc as bacc
import concourse.mybir as mybir
import concourse.tile as tile
from concourse.bass_utils import run_bass_kernel_spmd

# problem dims (hardcoded per spec)
B, C, H, W, D = 16, 512, 14, 14, 8000
P = H * W             # 196 positions per sample
BP = B * P            # 3136
KH = D // 2 + 1       # 4001 half-spectrum frequencies
KPAD = 4096           # padded to 8*512
NCORES = 8
KSL = KPAD // NCORES  # 512 freqs per core
NKT = KSL // 128      # 4 freq tiles per core
NCC = C // 128        # 4 contraction chunks (channels)
DH = 4001             # folded output dim (d and D-d share tables)
NDT = 32              # 32 d-tiles of 128 (4096 padded)

F32 = mybir.dt.float32
BF16 = mybir.dt.bfloat16
FP8 = mybir.dt.float8e4
DR = mybir.MatmulPerfMode.DoubleRow

E4 = ml_dtypes.float8_e4m3


def build_nc():
    nc = bacc.Bacc("TRN2", target_bir_lowering=False, debug=False)
    # x_d[p, v, cc, n]: channel cc*128+p, position n; v in (x8, xlo16, x8s)
    x_d = nc.dram_tensor("x", [128, 3, NCC, BP], FP8, kind="ExternalInput")
    # phi_d[p, v, kt, q, i, m, f]: Phi_m[(2q+i)*128+p, 512j+128kt+f]
    phi_d = nc.dram_tensor("phi", [128, 3, NKT, 2, 2, 4, 128], FP8,
                           kind="ExternalInput")
    # cc_d[t, kt, p, dt*128+f]: t0 = 196*wk*cos, t1 = -196*wk*sin
    cc_d = nc.dram_tensor("cc", [2, NKT, 128, NDT * 128], BF16,
                          kind="ExternalInput")
    out_d = nc.dram_tensor("out", [128, 2, NDT, B], BF16, kind="ExternalOutput")

    with tile.TileContext(nc) as tc:
        with (
            tc.tile_pool(name="xin", bufs=1) as x_pool,
            tc.tile_pool(name="phi", bufs=1) as phi_pool,
            tc.tile_pool(name="ccs", bufs=1) as cc_pool,
            tc.tile_pool(name="sbg", bufs=3) as sb_pool,
            tc.tile_pool(name="prd", bufs=2) as prod_pool,
            tc.tile_pool(name="com", bufs=2) as c_pool,
            tc.tile_pool(name="sht", bufs=4) as shat_pool,
            tc.tile_pool(name="stg", bufs=1) as st_pool,
            tc.tile_pool(name="gps", bufs=3, space="PSUM") as g_psum,
            tc.tile_pool(name="abs", bufs=1, space="PSUM") as ab_psum,
        ):
            # ---- PE warmup: ramp the clock through its p-states on dummy
            # matmuls while the first input DMAs are in flight
            warm = sb_pool.tile([128, 512], BF16, tag="warm")
            nc.vector.memset(warm[:], 0.0)
            wps = g_psum.tile([128, 4, 256], F32, tag="g", name="warmps")
            for w in range(9):
                nc.tensor.matmul(wps[:, w % 2, 0:252], lhsT=warm[:, 0:128],
                                 rhs=warm[:, 0:252], start=True, stop=True,
                                 skip_group_check=True)

            # ---- inputs, ordered so unit (kt0, s0) can start ASAP
            xt = x_pool.tile([128, 3, NCC, BP], FP8, tag="x")
            pt = phi_pool.tile([128, 3, NKT, 2, 2, 4, 128], FP8, tag="phi")
            nc.sync.dma_start(pt[:, :, 0], phi_d.ap()[:, :, 0])
            nc.sync.dma_start(xt[:, :, :, 0:196], x_d.ap()[:, :, :, 0:196])
            nc.sync.dma_start(xt[:, :, :, 196:392], x_d.ap()[:, :, :, 196:392])
            # x rest in 4-sample pieces, keeps ahead of kt0's consumption
            for c0 in range(392, BP, 784):
                c1 = min(c0 + 784, BP)
                nc.sync.dma_start(xt[:, :, :, c0:c1], x_d.ap()[:, :, :, c0:c1])
            nc.sync.dma_start(pt[:, :, 1:], phi_d.ap()[:, :, 1:])
            cct = {}
            for kt in range(NKT):
                for t in range(2):
                    ct = cc_pool.tile([128, NDT * 128], BF16, tag=f"cc{t}{kt}")
                    nc.sync.dma_start(ct[:], cc_d.ap()[t, kt])
                    cct[(t, kt)] = ct

            # ---- A/B accumulators (1 psum bank each, live whole kernel)
            apsum = ab_psum.tile([128, NDT, B], F32, tag="A")
            bpsum = ab_psum.tile([128, NDT, B], F32, tag="Bm")

            def stage3(kt):
                # A[dt] += ccRe[kt]^T Sre[kt];  B[dt] += ccIm[kt]^T Sim[kt]
                for t, ps in ((0, apsum), (1, bpsum)):
                    for dt in range(NDT):
                        nc.tensor.matmul(
                            ps[:, dt, :],
                            lhsT=cct[(t, kt)][:, dt * 128:(dt + 1) * 128],
                            rhs=shat[kt][:, t, :],
                            start=(kt == 0 and dt == 0),
                            stop=(kt == NKT - 1 and dt == NDT - 1),
                            skip_group_check=True,
                        )

            shat = [shat_pool.tile([128, 2, B], BF16, tag=f"shat{kt}",
                                   name=f"shat{kt}")
                    for kt in range(NKT)]
            shatf = [shat_pool.tile([128, 2, B], F32, tag=f"shatf{kt}",
                                    name=f"shatf{kt}")
                     for kt in range(NKT)]

            # ---- main loop: kt-major, 16 samples each; stage3(kt) slotted
            # a few units into kt+1 so Shat(kt) has drained the vector pipe
            for kt in range(NKT):
                for s in range(B):
                    g = g_psum.tile([128, 4, 256], F32, tag="g",
                                    name=f"g{kt}_{s}")
                    n0 = s * P
                    for m in range(4):
                        for v in range(3):  # x8@p8, xlo16@p8s, x8s@plo16
                            xv = pv = v
                            for q in range(2):
                                nc.tensor.matmul(
                                    g[:, m, 0:P],
                                    lhsT=pt[:, pv, kt, q, :, m, :],
                                    rhs=xt[:, xv, 2 * q:2 * q + 2, n0:n0 + P],
                                    start=(m % 2 == 0 and v == 0 and q == 0),
                                    stop=(m % 2 == 1 and v == 2 and q == 1),
                                    perf_mode=DR,
                                    skip_group_check=True,
                                )
                    # ACT: psum f32 -> sbuf bf16, 2 samples per group
                    if s % 2 == 0:
                        sbt = sb_pool.tile([128, 4, 2, P], BF16, tag="sb",
                                           name=f"sb{kt}_{s}")
                    nc.scalar.copy(sbt[:, :, s % 2, :], g[:, :, 0:P])
                    if s % 2 == 1:
                        # DVE: (RR, II) and (RI, IR) via strided muls
                        pr = prod_pool.tile([128, 4, 2, P], BF16, tag="pr",
                                            name=f"pr{kt}_{s}")
                        nc.vector.tensor_mul(pr[:, 0:2], sbt[:, 0:2],
                                             sbt[:, 2:4])
                        nc.vector.tensor_mul(pr[:, 2], sbt[:, 0], sbt[:, 3])
                        nc.vector.tensor_mul(pr[:, 3], sbt[:, 1], sbt[:, 2])
                        # Pool: Re = RR - II, Im = RI + IR
                        # (cm free-dim padded to 256 so the AP optimizer
                        # cannot merge dims — pool's window must stay 196)
                        cm = c_pool.tile([128, 2, 2, 256], BF16, tag="cm",
                                         name=f"cm{kt}_{s}")
                        nc.gpsimd.tensor_sub(cm[:, 0, :, 0:P], pr[:, 0],
                                             pr[:, 1])
                        nc.gpsimd.tensor_add(cm[:, 1, :, 0:P], pr[:, 2],
                                             pr[:, 3])
                        # DVE: sum over positions -> Shat columns (s-1, s)
                        nc.vector.tensor_reduce(
                            shatf[kt][:, :, s - 1:s + 1], cm[:, :, :, 0:P],
                            axis=mybir.AxisListType.X, op=mybir.AluOpType.add)
                        if s == B - 1:
                            nc.scalar.copy(shat[kt][:], shatf[kt][:])
                    if kt > 0 and s == 3:
                        stage3(kt - 1)
            stage3(NKT - 1)

            # ---- drain A/B -> bf16 -> HBM
            stage = st_pool.tile([128, 2, NDT, B], BF16, tag="stage")
            nc.scalar.copy(stage[:, 0], apsum[:])
            nc.scalar.copy(stage[:, 1], bpsum[:])
            nc.scalar.dma_start(out_d.ap(), stage[:])

    nc.compile()
    return nc


def make_constants(S1, S2):
    """Host-side constant prep from the sketch matrices (per-core slices)."""
    S1 = np.asarray(S1, np.float64)
    S2 = np.asarray(S2, np.float64)
    Phi = np.zeros((4, C, KPAD), np.float32)
    for i, S in enumerate((S1, S2)):
        F = np.fft.fft(S, axis=1)[:, :KH]
        Phi[2 * i, :, :KH] = F.real.astype(np.float32)
        Phi[2 * i + 1, :, :KH] = F.imag.astype(np.float32)

    # fp8 3-term split of Phi
    p8 = Phi.astype(E4)
    plo16 = ((Phi - p8.astype(np.float32)) * 16).astype(E4)
    p8s = (p8.astype(np.float32) / 16).astype(E4)

    # phi layout [128, v, kt, q, i, m, f]; v = (p8, p8s, plo16)
    # element [p, v, kt, q, i, m, f] = PhiV[m, (2q+i)*128+p, 512j+128kt+f]
    phis = []
    stack = np.stack([p8, p8s, plo16], 0)  # [v, m, c, k]
    arr = stack.reshape(3, 4, 2, 2, 128, NCORES, NKT, 128)  # v m q i p j kt f
    for j in range(NCORES):
        a = arr[:, :, :, :, :, j]  # [v, m, q, i, p, kt, f]
        a = np.transpose(a, (4, 0, 5, 2, 3, 1, 6))  # p v kt q i m f
        phis.append(np.ascontiguousarray(a))

    # inverse half-DFT tables, d/D-d folded
    k = np.arange(KPAD, dtype=np.float64)
    wk = np.where((k == 0) | (k == D // 2), 1.0, 2.0) / D
    wk[KH:] = 0.0
    ang = 2.0 * np.pi * np.outer(k, np.arange(DH, dtype=np.float64)) / D
    Cst = np.zeros((2, KPAD, NDT * 128), np.float32)
    Cst[0, :, :DH] = wk[:, None] * np.cos(ang)
    Cst[1, :, :DH] = -wk[:, None] * np.sin(ang)
    Cst = Cst.astype(ml_dtypes.bfloat16)
    ccs = [np.ascontiguousarray(
        Cst.reshape(2, NCORES, NKT, 128, NDT * 128)[:, j]) for j in range(NCORES)]
    return phis, ccs


def prep_x(x):
    """[B, C, H, W] -> [128, 3, cc, BP] fp8 (x8, xlo16, x8s)."""
    xr = np.ascontiguousarray(
        np.asarray(x, np.float32).reshape(B, C, P).transpose(1, 0, 2)
    ).reshape(C, BP)
    x8 = xr.astype(E4)
    xlo16 = ((xr - x8.astype(np.float32)) * 16).astype(E4)
    x8s = (x8.astype(np.float32) / 16).astype(E4)
    out = np.empty((128, 3, NCC, BP), E4)
    for v, t in enumerate((x8, xlo16, x8s)):
        out[:, v] = t.reshape(NCC, 128, BP).transpose(1, 0, 2)
    return out


def unshard(parts):
    """Sum per-core [128, 2, NDT, B] bf16 partials -> [B, D] f32."""
    acc = np.zeros((2, NDT * 128, B), np.float32)
    for r in parts:
        a = np.asarray(r, np.float32)  # [128, 2, NDT, B]
        acc += a.transpose(1, 2, 0, 3).reshape(2, NDT * 128, B)
    A, Bm = acc[0], acc[1]
    out = np.zeros((D, B), np.float32)
    out[:KH] = A[:KH] + Bm[:KH]
    out[KH:] = (A[1:4000] - Bm[1:4000])[::-1]
    return np.ascontiguousarray(out.T)


_CACHE = {}


def kernel(x, S1, S2):
    x = np.asarray(x)
    if "k" not in _CACHE:
        phis, ccs = make_constants(np.asarray(S1), np.asarray(S2))
        _CACHE["k"] = (build_nc(), phis, ccs)
    nc, phis, ccs = _CACHE["k"]

    xp = prep_x(x)
    in_maps = [{"x": xp, "phi": phis[j], "cc": ccs[j]} for j in range(NCORES)]
    res = run_bass_kernel_spmd(nc, in_maps, list(range(NCORES)))
    return unshard([r["out"] for r in res.results]).astype(x.dtype)
